# revision 29
# baseline (speedup 1.0000x reference)
"""Trainium2 Bass kernel for nn_ContextDrivingForce (dense MLP, 3 fused layers).

Math (per token row, D=896):
    u_proj = u @ W_a.T + b_a
    alpha  = sigmoid(sum(h * u_proj) / sqrt(D))
    u_att  = alpha * u
    g      = sigmoid([h, u_att] @ W_g.T + b_g)
    u_gate = g * u_att
    out    = gelu([h, u_gate, h*u_gate] @ W_f.T + b_f)        (exact erf gelu)

Distribution: data-parallel over the token axis across 8 NeuronCores,
weights replicated. All device tensors are feature-major ([D, tokens]);
the host transposes inputs/weights and the final output, so the device
performs no transposes at all.

Both sigmoids are computed as tanh ((sigmoid(x) = (tanh(x/2)+1)/2)) so that
every activation (tanh, gelu) lives in the single `gelu_and_others` ACT
table set -- no table reloads.  The 1/2 factors are folded into host-side
weight scaling:
    ua' := (tanh(logit/2)+1) * u          = 2*u_att     -> W_g[:,D:] *= 1/2
    ug' := (tanh(z2/2)+1) * ua'           = 4*u_gate    -> W_f[:,D:2D] *= 1/4
    hu' := h * ug'                        = 4*h*u_gate  -> W_f[:,2D:] *= 1/4
"""

import math
import sys
from contextlib import ExitStack

for _p in ("/root/.axon_site", "/root/.axon_site/_ro/trn_rl_repo"):
    if _p not in sys.path:
        sys.path.append(_p)

import ml_dtypes
import numpy as np

import concourse.bass as bass
import concourse.mybir as mybir
import concourse.tile as tile
from concourse import bacc, bass_isa
from concourse.bass_utils import run_bass_kernel_spmd

P = 128
D = 896
KD = D // P  # 7 feature tiles
N_TOK = 16384
N_CORES = 8
NPC = N_TOK // N_CORES  # 2048 tokens per core

F32 = mybir.dt.float32
AF = mybir.ActivationFunctionType
ALU = mybir.AluOpType


def build_nc(npc=NPC, T=512, mode="bf16", mm_bufs=6, act_bufs=None, gelu_native=True):
    if act_bufs is None:
        act_bufs = 2 if mode == "bf16" else 1
    """Build the single-core Bass program (same program runs SPMD on all cores)."""
    if mode == "bf16":
        cdt = mybir.dt.bfloat16
        mdt = mybir.dt.bfloat16
    elif mode == "fp32r":
        cdt = F32
        mdt = mybir.dt.float32r
    elif mode == "fp32":
        cdt = F32
        mdt = F32
    else:
        raise ValueError(mode)

    n_chunks = npc // T
    assert n_chunks * T == npc

    nc = bacc.Bacc()
    # inputs are chunk-major [P, n_chunks, KD, T]: each chunk DMA reads
    # 7KB contiguous per partition (near-peak DMA efficiency)
    hT_d = nc.declare_dram_parameter("hT", [P, npc // T, KD, T], cdt, isOutput=False)
    uT_d = nc.declare_dram_parameter("uT", [P, npc // T, KD, T], cdt, isOutput=False)
    # weights as three DMAs, W_a first, so layer-1 matmuls can start while
    # W_g / W_f are still in flight (HWDGE ring is FIFO in trigger order)
    wa_d = nc.declare_dram_parameter("wa", [P, KD, D], cdt, isOutput=False)
    wg_d = nc.declare_dram_parameter("wg", [P, 2 * KD, D], cdt, isOutput=False)
    wf_d = nc.declare_dram_parameter("wf", [P, 3 * KD, D], cdt, isOutput=False)
    bias_d = nc.declare_dram_parameter("biasp", [P, 3 * KD], F32, isOutput=False)
    gT_d = nc.declare_dram_parameter("gT", [D, npc], F32, isOutput=True)

    inv_sqrt_d = 1.0 / math.sqrt(D)

    def mm(ps, lhsT, rhs, start, stop):
        if mdt != cdt:
            lhsT = lhsT.bitcast(mdt)
            rhs = rhs.bitcast(mdt)
        nc.tensor.matmul(ps, lhsT=lhsT, rhs=rhs, start=start, stop=stop)

    with tile.TileContext(nc) as tc, ExitStack() as ctx:
        wp = ctx.enter_context(tc.tile_pool(name="weights", bufs=1))
        hp = ctx.enter_context(tc.tile_pool(name="hp", bufs=act_bufs))
        up = ctx.enter_context(tc.tile_pool(name="up", bufs=act_bufs))
        uap = ctx.enter_context(tc.tile_pool(name="uap", bufs=act_bufs))
        ugp = ctx.enter_context(tc.tile_pool(name="ugp", bufs=act_bufs))
        hup = ctx.enter_context(tc.tile_pool(name="hup", bufs=act_bufs))
        sp = ctx.enter_context(tc.tile_pool(name="small", bufs=3))
        op = ctx.enter_context(tc.tile_pool(name="outp", bufs=3))
        pp = ctx.enter_context(tc.tile_pool(name="psum", bufs=1, space="PSUM"))

        bias_sb = wp.tile([P, 3 * KD], F32, name="biasp")
        nc.sync.dma_start(bias_sb, bias_d[:, :])

        def load_chunk(c):
            h_sb = hp.tile([P, KD, T], cdt, name=f"h{c}", tag="h")
            nc.sync.dma_start(h_sb, hT_d[:, c])
            u_sb = up.tile([P, KD, T], cdt, name=f"u{c}", tag="u")
            nc.sync.dma_start(u_sb, uT_d[:, c])
            return h_sb, u_sb

        # Prelude: stream W_a + chunk-0 inputs per k-tile, interleaved in DMA
        # FIFO order, so layer-1 matmul k=0 can start after ~2 small DMAs
        # instead of waiting for all prelude bytes.
        wa_sb = wp.tile([P, KD, D], cdt, name="wa")
        h0_sb = hp.tile([P, KD, T], cdt, name="h0", tag="h")
        u0_sb = up.tile([P, KD, T], cdt, name="u0", tag="u")
        for k in range(KD):
            nc.sync.dma_start(wa_sb[:, k], wa_d[:, k])
            nc.sync.dma_start(u0_sb[:, k], uT_d[:, 0, k])
        for k in range(KD):
            nc.sync.dma_start(h0_sb[:, k], hT_d[:, 0, k])
        chunk0 = (h0_sb, u0_sb)
        # W_g / W_f staged to match consumption order (h-side k-tiles first)
        wg_sb = wp.tile([P, 2 * KD, D], cdt, name="wg")
        nc.sync.dma_start(wg_sb[:, :KD], wg_d[:, :KD])
        nc.sync.dma_start(wg_sb[:, KD:], wg_d[:, KD:])
        wf_sb = wp.tile([P, 3 * KD, D], cdt, name="wf")
        for j in range(3):
            nc.sync.dma_start(wf_sb[:, j * KD:(j + 1) * KD],
                              wf_d[:, j * KD:(j + 1) * KD])

        M_GROUPS = [list(range(0, 4)), list(range(4, KD))]

        for c in range(n_chunks):
            cs = bass.ds(c * T, T)
            h_sb, u_sb = chunk0 if c == 0 else load_chunk(c)

            # ---- layer 1: u_proj = u @ W_a.T (feature-major), fused logit
            # reduce. Grouped-k-major: the k-loop is innermost across a group
            # of <=4 M-tiles so compute starts as soon as k-tile 0 arrives.
            # The per-token logit is accumulated on DVE in fp32 (acc), then
            # summed across partitions (with broadcast) by one GPSIMD
            # partition_all_reduce -- no PE matmuls in the reduction.
            acc = sp.tile([P, T], F32, name=f"acc{c}", tag="acc", bufs=2)
            for grp in M_GROUPS:
                pss = {m: pp.tile([P, T], F32, name=f"ps1_{c}_{m}", tag="mm",
                                  bufs=mm_bufs) for m in grp}
                for k in range(KD):
                    for m in grp:
                        mm(pss[m], wa_sb[:, k, m * P:(m + 1) * P], u_sb[:, k, :],
                           start=(k == 0), stop=(k == KD - 1))
                for m in grp:
                    # (u_proj + b_a) * h for this feature block (fused DVE op)
                    if m == 0:
                        nc.vector.scalar_tensor_tensor(
                            out=acc, in0=pss[m], scalar=bias_sb[:, 0:1],
                            in1=h_sb[:, 0, :], op0=ALU.add, op1=ALU.mult)
                    else:
                        tmp = sp.tile([P, T], F32, name=f"tmp{c}_{m}",
                                      tag="tmp", bufs=2)
                        nc.vector.scalar_tensor_tensor(
                            out=tmp, in0=pss[m], scalar=bias_sb[:, m:m + 1],
                            in1=h_sb[:, m, :], op0=ALU.add, op1=ALU.mult)
                        nc.vector.tensor_add(out=acc, in0=acc, in1=tmp)

            # logit (summed over all partitions, broadcast to all of them)
            albc = sp.tile([P, T], F32, name=f"albc{c}", tag="albc", bufs=2)
            nc.gpsimd.partition_all_reduce(albc, acc, channels=P,
                                           reduce_op=bass_isa.ReduceOp.add)
            # alpha' = tanh(logit / (2 sqrt(D))) = 2*sigmoid(logit) - 1
            t1b = sp.tile([P, T], cdt, name=f"t1b{c}", tag="t1b", bufs=2)
            nc.scalar.activation(t1b, albc, AF.Tanh, scale=inv_sqrt_d * 0.5)

            # ua' = (alpha'+1) * u = 2 * u_att
            ua_sb = uap.tile([P, KD, T], cdt, name=f"ua{c}", tag="ua")
            for k in range(KD):
                nc.vector.scalar_tensor_tensor(
                    out=ua_sb[:, k, :], in0=t1b, scalar=1.0, in1=u_sb[:, k, :],
                    op0=ALU.add, op1=ALU.mult)

            # ---- layer 2: z2 = [h, u_att] @ W_g.T ; ug' = (tanh((z2+b)/2)+1)*ua'
            ug_sb = ugp.tile([P, KD, T], cdt, name=f"ug{c}", tag="ug")
            for grp in M_GROUPS:
                pss = {m: pp.tile([P, T], F32, name=f"ps2_{c}_{m}", tag="mm",
                                  bufs=mm_bufs) for m in grp}
                for k in range(2 * KD):
                    rhs = h_sb[:, k, :] if k < KD else ua_sb[:, k - KD, :]
                    for m in grp:
                        mm(pss[m], wg_sb[:, k, m * P:(m + 1) * P], rhs,
                           start=(k == 0), stop=(k == 2 * KD - 1))
                for m in grp:
                    t2 = sp.tile([P, T], cdt, name=f"t2_{c}_{m}", tag="t2")
                    nc.scalar.activation(t2, pss[m], AF.Tanh,
                                         bias=bias_sb[:, KD + m:KD + m + 1],
                                         scale=0.5)
                    nc.vector.scalar_tensor_tensor(
                        out=ug_sb[:, m, :], in0=t2, scalar=1.0,
                        in1=ua_sb[:, m, :], op0=ALU.add, op1=ALU.mult)

            # hu' = h * ug' (= 4*h*u_gate; the 1/4 is folded into W_f cols)
            hu_sb = hup.tile([P, KD, T], cdt, name=f"hu{c}", tag="hu")
            for k in range(KD):
                nc.vector.tensor_mul(out=hu_sb[:, k, :], in0=h_sb[:, k, :],
                                     in1=ug_sb[:, k, :])

            # ---- layer 3: out = gelu([h, ug', hu'] @ W_f'.T + b_f)
            for m in range(KD):
                ps = pp.tile([P, T], F32, name=f"ps3_{c}_{m}", tag="mm", bufs=mm_bufs)
                for k in range(3 * KD):
                    if k < KD:
                        rhs = h_sb[:, k, :]
                    elif k < 2 * KD:
                        rhs = ug_sb[:, k - KD, :]
                    else:
                        rhs = hu_sb[:, k - 2 * KD, :]
                    mm(ps, wf_sb[:, k, m * P:(m + 1) * P], rhs,
                       start=(k == 0), stop=(k == 3 * KD - 1))
                outp = op.tile([P, T], F32, name=f"o{c}_{m}", tag="out")
                nc.scalar.activation(outp, ps,
                                     AF.Gelu if gelu_native else AF.Identity,
                                     bias=bias_sb[:, 2 * KD + m:2 * KD + m + 1],
                                     scale=1.0)
                # output stores ride the ACT HWDGE ring so they never block
                # the input-load FIFO on the SP ring
                nc.scalar.dma_start(gT_d[m * P:(m + 1) * P, cs], outp)
    nc.compile()  # bacc passes: split >1-wait instrs onto EventSemaphores, etc.
    return nc


def prep_inputs(h_t, u_t, W_a_w, W_a_b, W_g_w, W_g_b, W_f_w, W_f_b,
                npc=NPC, T=512, mode="bf16"):
    """Host-side layout prep: transpose to feature-major, fold tanh-trick
    scales into the weights, pack per-out-feature biases, shard tokens."""
    np_dt = ml_dtypes.bfloat16 if mode == "bf16" else np.float32

    h = np.asarray(h_t, np.float32)
    u = np.asarray(u_t, np.float32)
    Wa = np.asarray(W_a_w, np.float32)
    Wg = np.asarray(W_g_w, np.float32)
    Wf = np.asarray(W_f_w, np.float32)
    ba = np.asarray(W_a_b, np.float32)
    bg = np.asarray(W_g_b, np.float32)
    bf = np.asarray(W_f_b, np.float32)

    waT = Wa.T  # [in, out]
    wgT = np.concatenate([Wg[:, :D], Wg[:, D:] * 0.5], axis=1).T
    wfT = np.concatenate([Wf[:, :D], Wf[:, D:2 * D] * 0.25, Wf[:, 2 * D:] * 0.25],
                         axis=1).T

    def wpack(w):  # [K_in, D_out] -> [128, K_in/128, D_out]
        return np.ascontiguousarray(
            w.reshape(-1, P, D).transpose(1, 0, 2)).astype(np_dt)

    wa_p, wg_p, wf_p = wpack(waT), wpack(wgT), wpack(wfT)
    # bias pack: [128, 21] fp32; column m is out-features [m*128,(m+1)*128)
    # of b_a (layer1), 0.5*b_g (layer2 tanh arg), b_f (layer3)
    biasp = np.ascontiguousarray(
        np.concatenate([ba, 0.5 * bg, bf]).reshape(3 * KD, P).T).astype(np.float32)

    # chunk-major input pack: [P, n_chunks, KD, T] per core, so each chunk's
    # DMA is 7KB-contiguous per partition. From token-major [N, D]:
    # pack[p, c, k, t] = x[core*npc + c*T + t, k*128 + p]
    nch = npc // T

    def xpack(x, i):  # x [N, D] -> [P, nch, KD, T] for core i
        blk = x[i * npc:(i + 1) * npc]                    # [npc, D]
        blk = blk.reshape(nch, T, KD, P)                  # [c, t, k, p]
        return np.ascontiguousarray(
            blk.transpose(3, 0, 2, 1)).astype(np_dt)      # [p, c, k, t]

    n_cores = h.shape[0] // npc
    in_maps = []
    for i in range(n_cores):
        in_maps.append({
            "hT": xpack(h, i),
            "uT": xpack(u, i),
            "wa": wa_p, "wg": wg_p, "wf": wf_p, "biasp": biasp,
        })
    return in_maps


_NC_CACHE = {}


def _get_nc(npc=NPC, T=512, mode="bf16"):
    key = (npc, T, mode)
    if key not in _NC_CACHE:
        _NC_CACHE[key] = build_nc(npc=npc, T=T, mode=mode)
    return _NC_CACHE[key]


def run(inputs, npc=NPC, T=None, mode="bf16", trace=False, **kw):
    """Run the SPMD kernel; returns (full_output [N,D] fp32, BassKernelResults)."""
    if T is None:
        T = 512 if mode == "bf16" else 256
    nc = _get_nc(npc=npc, T=T, mode=mode)
    in_maps = prep_inputs(
        inputs["h_t"], inputs["u_t"], inputs["W_a_w"], inputs["W_a_b"],
        inputs["W_g_w"], inputs["W_g_b"], inputs["W_f_w"], inputs["W_f_b"],
        npc=npc, T=T, mode=mode)
    res = run_bass_kernel_spmd(nc, in_maps, list(range(len(in_maps))),
                               trace=trace, **kw)
    out = np.concatenate(
        [np.asarray(r["gT"], np.float32).T for r in res.results], axis=0)
    return out, res


def kernel(h_t, u_t, token_idx, u_all, W_a_w, W_a_b, W_g_w, W_g_b, W_f_w, W_f_b):
    # token_idx / u_all are unused by the reference math.
    inputs = {"h_t": h_t, "u_t": u_t, "W_a_w": W_a_w, "W_a_b": W_a_b,
              "W_g_w": W_g_w, "W_g_b": W_g_b, "W_f_w": W_f_w, "W_f_b": W_f_b}
    out, _ = run(inputs)
    return out


if __name__ == "__main__":
    # tiny smoke test through CoreSim is in test.py; direct run does HW.
    rng = np.random.default_rng(0)
    fake = {
        "h_t": rng.standard_normal((N_TOK, D), dtype=np.float32),
        "u_t": rng.standard_normal((N_TOK, D), dtype=np.float32),
        "W_a_w": rng.standard_normal((D, D), dtype=np.float32) * 0.02,
        "W_a_b": rng.standard_normal((D,), dtype=np.float32) * 0.02,
        "W_g_w": rng.standard_normal((D, 2 * D), dtype=np.float32) * 0.02,
        "W_g_b": rng.standard_normal((D,), dtype=np.float32) * 0.02,
        "W_f_w": rng.standard_normal((D, 3 * D), dtype=np.float32) * 0.02,
        "W_f_b": rng.standard_normal((D,), dtype=np.float32) * 0.02,
    }
    out, res = run(fake)
    print("out", out.shape, out.dtype, "exec_time_ns", res.exec_time_ns)


# revision 30
# speedup vs baseline: 1.0231x; 1.0231x over previous
"""Trainium2 Bass kernel for nn_ContextDrivingForce (dense MLP, 3 fused layers).

Math (per token row, D=896):
    u_proj = u @ W_a.T + b_a
    alpha  = sigmoid(sum(h * u_proj) / sqrt(D))
    u_att  = alpha * u
    g      = sigmoid([h, u_att] @ W_g.T + b_g)
    u_gate = g * u_att
    out    = gelu([h, u_gate, h*u_gate] @ W_f.T + b_f)        (exact erf gelu)

Distribution: data-parallel over the token axis across 8 NeuronCores,
weights replicated. All device tensors are feature-major ([D, tokens]);
the host transposes inputs/weights and the final output, so the device
performs no transposes at all.

Both sigmoids are computed as tanh ((sigmoid(x) = (tanh(x/2)+1)/2)) so that
every activation (tanh, gelu) lives in the single `gelu_and_others` ACT
table set -- no table reloads.  The 1/2 factors are folded into host-side
weight scaling:
    ua' := (tanh(logit/2)+1) * u          = 2*u_att     -> W_g[:,D:] *= 1/2
    ug' := (tanh(z2/2)+1) * ua'           = 4*u_gate    -> W_f[:,D:2D] *= 1/4
    hu' := h * ug'                        = 4*h*u_gate  -> W_f[:,2D:] *= 1/4
"""

import math
import sys
from contextlib import ExitStack

for _p in ("/root/.axon_site", "/root/.axon_site/_ro/trn_rl_repo"):
    if _p not in sys.path:
        sys.path.append(_p)

import ml_dtypes
import numpy as np

import concourse.bass as bass
import concourse.mybir as mybir
import concourse.tile as tile
from concourse import bacc
from concourse.bass_utils import run_bass_kernel_spmd

P = 128
D = 896
KD = D // P  # 7 feature tiles
N_TOK = 16384
N_CORES = 8
NPC = N_TOK // N_CORES  # 2048 tokens per core

F32 = mybir.dt.float32
AF = mybir.ActivationFunctionType
ALU = mybir.AluOpType


def build_nc(npc=NPC, T=512, mode="bf16", mm_bufs=5, act_bufs=None, gelu_native=True):
    if act_bufs is None:
        act_bufs = 2 if mode == "bf16" else 1
    """Build the single-core Bass program (same program runs SPMD on all cores)."""
    if mode == "bf16":
        cdt = mybir.dt.bfloat16
        mdt = mybir.dt.bfloat16
    elif mode == "fp32r":
        cdt = F32
        mdt = mybir.dt.float32r
    elif mode == "fp32":
        cdt = F32
        mdt = F32
    else:
        raise ValueError(mode)

    n_chunks = npc // T
    assert n_chunks * T == npc

    nc = bacc.Bacc()
    # inputs are chunk-major [P, n_chunks, KD, T]: each chunk DMA reads
    # 7KB contiguous per partition (near-peak DMA efficiency)
    hT_d = nc.declare_dram_parameter("hT", [P, npc // T, KD, T], cdt, isOutput=False)
    uT_d = nc.declare_dram_parameter("uT", [P, npc // T, KD, T], cdt, isOutput=False)
    # weights as three DMAs, W_a first, so layer-1 matmuls can start while
    # W_g / W_f are still in flight (HWDGE ring is FIFO in trigger order)
    wa_d = nc.declare_dram_parameter("wa", [P, KD, D], cdt, isOutput=False)
    wg_d = nc.declare_dram_parameter("wg", [P, 2 * KD, D], cdt, isOutput=False)
    wf_d = nc.declare_dram_parameter("wf", [P, 3 * KD, D], cdt, isOutput=False)
    bias_d = nc.declare_dram_parameter("biasp", [P, 3 * KD], F32, isOutput=False)
    gT_d = nc.declare_dram_parameter("gT", [D, npc], F32, isOutput=True)

    inv_sqrt_d = 1.0 / math.sqrt(D)

    def mm(ps, lhsT, rhs, start, stop):
        if mdt != cdt:
            lhsT = lhsT.bitcast(mdt)
            rhs = rhs.bitcast(mdt)
        nc.tensor.matmul(ps, lhsT=lhsT, rhs=rhs, start=start, stop=stop)

    with tile.TileContext(nc) as tc, ExitStack() as ctx:
        wp = ctx.enter_context(tc.tile_pool(name="weights", bufs=1))
        hp = ctx.enter_context(tc.tile_pool(name="hp", bufs=act_bufs))
        up = ctx.enter_context(tc.tile_pool(name="up", bufs=act_bufs))
        uap = ctx.enter_context(tc.tile_pool(name="uap", bufs=act_bufs))
        ugp = ctx.enter_context(tc.tile_pool(name="ugp", bufs=act_bufs))
        hup = ctx.enter_context(tc.tile_pool(name="hup", bufs=act_bufs))
        sp = ctx.enter_context(tc.tile_pool(name="small", bufs=3))
        op = ctx.enter_context(tc.tile_pool(name="outp", bufs=3))
        pp = ctx.enter_context(tc.tile_pool(name="psum", bufs=1, space="PSUM"))

        bias_sb = wp.tile([P, 3 * KD], F32, name="biasp")
        nc.sync.dma_start(bias_sb, bias_d[:, :])
        ones_col = wp.tile([P, 1], cdt, name="ones_col")
        nc.vector.memset(ones_col, 1.0)
        ones_row = wp.tile([1, P], cdt, name="ones_row")
        nc.vector.memset(ones_row, 1.0)

        def load_chunk(c):
            h_sb = hp.tile([P, KD, T], cdt, name=f"h{c}", tag="h")
            nc.sync.dma_start(h_sb, hT_d[:, c])
            u_sb = up.tile([P, KD, T], cdt, name=f"u{c}", tag="u")
            nc.sync.dma_start(u_sb, uT_d[:, c])
            return h_sb, u_sb

        # Prelude: stream W_a + chunk-0 inputs per k-tile, interleaved in DMA
        # FIFO order, so layer-1 matmul k=0 can start after ~2 small DMAs
        # instead of waiting for all prelude bytes.
        wa_sb = wp.tile([P, KD, D], cdt, name="wa")
        h0_sb = hp.tile([P, KD, T], cdt, name="h0", tag="h")
        u0_sb = up.tile([P, KD, T], cdt, name="u0", tag="u")
        for k in range(KD):
            nc.sync.dma_start(wa_sb[:, k], wa_d[:, k])
            nc.sync.dma_start(u0_sb[:, k], uT_d[:, 0, k])
            nc.sync.dma_start(h0_sb[:, k], hT_d[:, 0, k])
        chunk0 = (h0_sb, u0_sb)
        # W_g / W_f staged to match consumption order (h-side k-tiles first)
        wg_sb = wp.tile([P, 2 * KD, D], cdt, name="wg")
        nc.sync.dma_start(wg_sb[:, :KD], wg_d[:, :KD])
        nc.sync.dma_start(wg_sb[:, KD:], wg_d[:, KD:])
        wf_sb = wp.tile([P, 3 * KD, D], cdt, name="wf")
        for j in range(3):
            nc.sync.dma_start(wf_sb[:, j * KD:(j + 1) * KD],
                              wf_d[:, j * KD:(j + 1) * KD])

        M_GROUPS = [list(range(0, 4)), list(range(4, KD))]

        for c in range(n_chunks):
            cs = bass.ds(c * T, T)
            h_sb, u_sb = chunk0 if c == 0 else load_chunk(c)

            # ---- layer 1: u_proj = u @ W_a.T (feature-major), fused logit
            # reduce. Grouped-k-major: the k-loop is innermost across a group
            # of <=4 M-tiles so compute starts as soon as k-tile 0 arrives.
            red = pp.tile([1, T], F32, name=f"red{c}", tag="red", bufs=1)
            tmps = []
            for grp in M_GROUPS:
                pss = {m: pp.tile([P, T], F32, name=f"ps1_{c}_{m}", tag="mm",
                                  bufs=mm_bufs) for m in grp}
                for k in range(KD):
                    for m in grp:
                        mm(pss[m], wa_sb[:, k, m * P:(m + 1) * P], u_sb[:, k, :],
                           start=(k == 0), stop=(k == KD - 1))
                for m in grp:
                    # tmp = (u_proj + b_a) * h   (one fused DVE op)
                    tmp = sp.tile([P, T], cdt, name=f"tmp{c}_{m}", tag="tmp",
                                  bufs=KD)
                    nc.vector.scalar_tensor_tensor(
                        out=tmp, in0=pss[m], scalar=bias_sb[:, m:m + 1],
                        in1=h_sb[:, m, :], op0=ALU.add, op1=ALU.mult)
                    tmps.append(tmp)
            # partition-reduce the 7 tmp tiles into the logit row
            for m in range(KD):
                mm(red, ones_col, tmps[m], start=(m == 0), stop=(m == KD - 1))

            # alpha' = tanh(logit / (2 sqrt(D))) = 2*sigmoid(logit) - 1
            alpha = sp.tile([1, T], cdt, name=f"al{c}", tag="alpha", bufs=2)
            nc.scalar.activation(alpha, red, AF.Tanh, scale=inv_sqrt_d * 0.5)
            # broadcast across partitions with a rank-1 matmul
            ab = pp.tile([P, T], F32, name=f"ab{c}", tag="ab", bufs=2)
            mm(ab, ones_row, alpha, start=True, stop=True)

            # ua' = (alpha'+1) * u = 2 * u_att
            ua_sb = uap.tile([P, KD, T], cdt, name=f"ua{c}", tag="ua")
            for k in range(KD):
                nc.vector.scalar_tensor_tensor(
                    out=ua_sb[:, k, :], in0=ab, scalar=1.0, in1=u_sb[:, k, :],
                    op0=ALU.add, op1=ALU.mult)

            # ---- layer 2: z2 = [h, u_att] @ W_g.T ; ug' = (tanh((z2+b)/2)+1)*ua'
            ug_sb = ugp.tile([P, KD, T], cdt, name=f"ug{c}", tag="ug")
            for grp in M_GROUPS:
                pss = {m: pp.tile([P, T], F32, name=f"ps2_{c}_{m}", tag="mm",
                                  bufs=mm_bufs) for m in grp}
                for k in range(2 * KD):
                    rhs = h_sb[:, k, :] if k < KD else ua_sb[:, k - KD, :]
                    for m in grp:
                        mm(pss[m], wg_sb[:, k, m * P:(m + 1) * P], rhs,
                           start=(k == 0), stop=(k == 2 * KD - 1))
                for m in grp:
                    t2 = sp.tile([P, T], cdt, name=f"t2_{c}_{m}", tag="t2")
                    nc.scalar.activation(t2, pss[m], AF.Tanh,
                                         bias=bias_sb[:, KD + m:KD + m + 1],
                                         scale=0.5)
                    nc.vector.scalar_tensor_tensor(
                        out=ug_sb[:, m, :], in0=t2, scalar=1.0,
                        in1=ua_sb[:, m, :], op0=ALU.add, op1=ALU.mult)

            # hu' = h * ug' (= 4*h*u_gate; the 1/4 is folded into W_f cols)
            hu_sb = hup.tile([P, KD, T], cdt, name=f"hu{c}", tag="hu")
            for k in range(KD):
                nc.vector.tensor_mul(out=hu_sb[:, k, :], in0=h_sb[:, k, :],
                                     in1=ug_sb[:, k, :])

            # ---- layer 3: out = gelu([h, ug', hu'] @ W_f'.T + b_f)
            for m in range(KD):
                ps = pp.tile([P, T], F32, name=f"ps3_{c}_{m}", tag="mm", bufs=mm_bufs)
                for k in range(3 * KD):
                    if k < KD:
                        rhs = h_sb[:, k, :]
                    elif k < 2 * KD:
                        rhs = ug_sb[:, k - KD, :]
                    else:
                        rhs = hu_sb[:, k - 2 * KD, :]
                    mm(ps, wf_sb[:, k, m * P:(m + 1) * P], rhs,
                       start=(k == 0), stop=(k == 3 * KD - 1))
                outp = op.tile([P, T], F32, name=f"o{c}_{m}", tag="out")
                nc.scalar.activation(outp, ps,
                                     AF.Gelu if gelu_native else AF.Identity,
                                     bias=bias_sb[:, 2 * KD + m:2 * KD + m + 1],
                                     scale=1.0)
                # output stores ride the ACT HWDGE ring so they never block
                # the input-load FIFO on the SP ring
                nc.scalar.dma_start(gT_d[m * P:(m + 1) * P, cs], outp)
    nc.compile()  # bacc passes: split >1-wait instrs onto EventSemaphores, etc.
    return nc


def prep_inputs(h_t, u_t, W_a_w, W_a_b, W_g_w, W_g_b, W_f_w, W_f_b,
                npc=NPC, T=512, mode="bf16"):
    """Host-side layout prep: transpose to feature-major, fold tanh-trick
    scales into the weights, pack per-out-feature biases, shard tokens."""
    np_dt = ml_dtypes.bfloat16 if mode == "bf16" else np.float32

    h = np.asarray(h_t, np.float32)
    u = np.asarray(u_t, np.float32)
    Wa = np.asarray(W_a_w, np.float32)
    Wg = np.asarray(W_g_w, np.float32)
    Wf = np.asarray(W_f_w, np.float32)
    ba = np.asarray(W_a_b, np.float32)
    bg = np.asarray(W_g_b, np.float32)
    bf = np.asarray(W_f_b, np.float32)

    waT = Wa.T  # [in, out]
    wgT = np.concatenate([Wg[:, :D], Wg[:, D:] * 0.5], axis=1).T
    wfT = np.concatenate([Wf[:, :D], Wf[:, D:2 * D] * 0.25, Wf[:, 2 * D:] * 0.25],
                         axis=1).T

    def wpack(w):  # [K_in, D_out] -> [128, K_in/128, D_out]
        return np.ascontiguousarray(
            w.reshape(-1, P, D).transpose(1, 0, 2)).astype(np_dt)

    wa_p, wg_p, wf_p = wpack(waT), wpack(wgT), wpack(wfT)
    # bias pack: [128, 21] fp32; column m is out-features [m*128,(m+1)*128)
    # of b_a (layer1), 0.5*b_g (layer2 tanh arg), b_f (layer3)
    biasp = np.ascontiguousarray(
        np.concatenate([ba, 0.5 * bg, bf]).reshape(3 * KD, P).T).astype(np.float32)

    # chunk-major input pack: [P, n_chunks, KD, T] per core, so each chunk's
    # DMA is 7KB-contiguous per partition. From token-major [N, D]:
    # pack[p, c, k, t] = x[core*npc + c*T + t, k*128 + p]
    nch = npc // T

    def xpack(x, i):  # x [N, D] -> [P, nch, KD, T] for core i
        blk = x[i * npc:(i + 1) * npc]                    # [npc, D]
        blk = blk.reshape(nch, T, KD, P)                  # [c, t, k, p]
        return np.ascontiguousarray(
            blk.transpose(3, 0, 2, 1)).astype(np_dt)      # [p, c, k, t]

    n_cores = h.shape[0] // npc
    in_maps = []
    for i in range(n_cores):
        in_maps.append({
            "hT": xpack(h, i),
            "uT": xpack(u, i),
            "wa": wa_p, "wg": wg_p, "wf": wf_p, "biasp": biasp,
        })
    return in_maps


_NC_CACHE = {}


def _get_nc(npc=NPC, T=512, mode="bf16"):
    key = (npc, T, mode)
    if key not in _NC_CACHE:
        _NC_CACHE[key] = build_nc(npc=npc, T=T, mode=mode)
    return _NC_CACHE[key]


def run(inputs, npc=NPC, T=None, mode="bf16", trace=False, **kw):
    """Run the SPMD kernel; returns (full_output [N,D] fp32, BassKernelResults)."""
    if T is None:
        T = 512 if mode == "bf16" else 256
    nc = _get_nc(npc=npc, T=T, mode=mode)
    in_maps = prep_inputs(
        inputs["h_t"], inputs["u_t"], inputs["W_a_w"], inputs["W_a_b"],
        inputs["W_g_w"], inputs["W_g_b"], inputs["W_f_w"], inputs["W_f_b"],
        npc=npc, T=T, mode=mode)
    res = run_bass_kernel_spmd(nc, in_maps, list(range(len(in_maps))),
                               trace=trace, **kw)
    out = np.concatenate(
        [np.asarray(r["gT"], np.float32).T for r in res.results], axis=0)
    return out, res


def kernel(h_t, u_t, token_idx, u_all, W_a_w, W_a_b, W_g_w, W_g_b, W_f_w, W_f_b):
    # token_idx / u_all are unused by the reference math.
    inputs = {"h_t": h_t, "u_t": u_t, "W_a_w": W_a_w, "W_a_b": W_a_b,
              "W_g_w": W_g_w, "W_g_b": W_g_b, "W_f_w": W_f_w, "W_f_b": W_f_b}
    out, _ = run(inputs)
    return out


if __name__ == "__main__":
    # tiny smoke test through CoreSim is in test.py; direct run does HW.
    rng = np.random.default_rng(0)
    fake = {
        "h_t": rng.standard_normal((N_TOK, D), dtype=np.float32),
        "u_t": rng.standard_normal((N_TOK, D), dtype=np.float32),
        "W_a_w": rng.standard_normal((D, D), dtype=np.float32) * 0.02,
        "W_a_b": rng.standard_normal((D,), dtype=np.float32) * 0.02,
        "W_g_w": rng.standard_normal((D, 2 * D), dtype=np.float32) * 0.02,
        "W_g_b": rng.standard_normal((D,), dtype=np.float32) * 0.02,
        "W_f_w": rng.standard_normal((D, 3 * D), dtype=np.float32) * 0.02,
        "W_f_b": rng.standard_normal((D,), dtype=np.float32) * 0.02,
    }
    out, res = run(fake)
    print("out", out.shape, out.dtype, "exec_time_ns", res.exec_time_ns)
